# revision 1
# baseline (speedup 1.0000x reference)
"""Trainium2 Bass kernel for nn_EnergyFunction (dense transformer block).

Reference math (B=2, S=2048, D=1024, H=8 heads, hd=128):
    K  = x @ Wk.T            [B,S,D] -> heads [B,H,S,hd]
    V  = x @ Wv.T
    E  = (K K^T)/sqrt(hd)    per head, causal mask (q >= k allowed)
    P  = softmax(-E, axis=k)
    O  = P @ V               -> [B,S,D]
    out = (O + x @ Wself.T) @ Wout.T

Sharding (8 cores): core c -> batch b=c//4, head pair hp=c%4 (heads 2hp,2hp+1,
dims ds=[256*hp, 256*hp+256)).  Each core computes
    partial_c = (O_heads + x @ Wself.T[:,ds]) @ Wout.T[ds,:]   [S, D]
and the host sums the 4 partials per batch (row/column-parallel Wout split).

On-core layout trick: all attention tensors are kept "transposed"
(k or head-dim on partitions, q on free dim).  E is symmetric, so score
tiles are computed directly in (k-part, q-free) orientation by swapping
matmul operands -- no on-chip transposes are needed anywhere.  The softmax
denominator l_q = sum_k P[q,k] is taken with a ones-vector matmul
(lhsT=ones[128,1]) accumulated alongside the P@V matmuls, and 1/l is
broadcast across partitions with a K=1 ones matmul.  Softmax max-subtraction
is skipped: |E|/sqrt(hd) <= ~11 for this distribution, exp() is safe in f32.

Matmuls run in float32r (full PE rate at free-dim>=256, ~1.5e-4 rel err).
The 1/sqrt(hd) scaling is folded into Wk on the host (hd**-0.25 on both
operands of K K^T).
"""

import os
import sys

import numpy as np

if "/opt/trn_rl_repo" not in sys.path:
    sys.path.insert(0, "/opt/trn_rl_repo")

import concourse.bass as bass
import concourse.mybir as mybir
import concourse.tile as tile
from concourse.bass import ts
from concourse.bass_utils import run_bass_kernel_spmd

B, S, D = 2, 2048, 1024
H = 8
HD = D // H          # 128 head dim
HPC = 2              # heads per core
DS = HPC * HD        # 256 dims per core
N_CORES = 8
P = 128              # partitions
QC = 512             # q chunk width
NQC = S // QC        # 4 q chunks
NKT = S // P         # 16 k tiles
NDC = D // P         # 8 contraction chunks over D

F32 = mybir.dt.float32
F32R = mybir.dt.float32r
EXP = mybir.ActivationFunctionType.Exp


def _legalize_waits(nc):
    """This toolchain's walrus rejects >1 semaphore wait on several
    instruction structs (Drain/CTRL allows none, Matmult/Ldweights S3_LW
    allows one).  Hoist excess waits onto same-engine NOPs placed
    immediately before the offending instruction."""
    for blk in nc.main_func.blocks:
        insts = blk.instructions
        new = []
        changed = False
        for ins in insts:
            si = ins.sync_info
            if si is not None and si.on_wait:
                allow = 0 if ins.opcode == "Drain" else 1
                waits = list(si.on_wait)
                if len(waits) > allow:
                    cut = len(waits) - allow
                    for k, w in enumerate(waits[:cut]):
                        nop = mybir.InstNoOp(
                            name=f"{ins.name}-wsplit{k}", engine=ins.engine
                        )
                        nop.sync_info = mybir.SyncInfo(on_wait=[w], on_update=[])
                        new.append(nop)
                    ins.sync_info = mybir.SyncInfo(
                        on_wait=waits[cut:], on_update=list(si.on_update)
                    )
                    changed = True
            new.append(ins)
        if changed:
            blk.instructions = new


def _build():
    nc = bass.Bass()

    xT = nc.dram_tensor("xT", [D, S], F32R, kind="ExternalInput")
    wkT = nc.dram_tensor("wkT", [D, DS], F32R, kind="ExternalInput")
    wvT = nc.dram_tensor("wvT", [D, DS], F32R, kind="ExternalInput")
    wselfT = nc.dram_tensor("wselfT", [D, DS], F32R, kind="ExternalInput")
    woutT = nc.dram_tensor("woutT", [DS, D], F32R, kind="ExternalInput")
    ones_k = nc.dram_tensor("ones_k", [P, 1], F32R, kind="ExternalInput")
    ones_1 = nc.dram_tensor("ones_1", [1, P], F32R, kind="ExternalInput")
    mask01 = nc.dram_tensor("mask01", [P, P], F32, kind="ExternalInput")
    part = nc.dram_tensor("part", [S, D], F32, kind="ExternalOutput")

    with tile.TileContext(nc) as tc:
        with (
            tc.tile_pool(name="persist", bufs=1) as pp,
            tc.tile_pool(name="pt_pool", bufs=3) as pt_pool,
            tc.tile_pool(name="rb_pool", bufs=2) as rb_pool,
            tc.tile_pool(name="out_pool", bufs=3) as out_pool,
            tc.tile_pool(name="ps_proj", bufs=2, space="PSUM") as ps_proj,
            tc.tile_pool(name="ps_e", bufs=2, space="PSUM") as ps_e,
            tc.tile_pool(name="ps_ot", bufs=2, space="PSUM") as ps_ot,
            tc.tile_pool(name="ps_l", bufs=1, space="PSUM") as ps_l,
            tc.tile_pool(name="ps_bc", bufs=1, space="PSUM") as ps_bc,
        ):
            # ---- persistent SBUF tensors ----
            xT_sb = pp.tile([P, NDC, S], F32R, name="xT_sb")
            wkT_sb = pp.tile([P, NDC, DS], F32R, name="wkT_sb")
            wvT_sb = pp.tile([P, NDC, DS], F32R, name="wvT_sb")
            wselfT_sb = pp.tile([P, NDC, DS], F32R, name="wselfT_sb")
            woutT_sb = pp.tile([P, HPC, D], F32R, name="woutT_sb")
            kt_sb = pp.tile([P, HPC, S], F32R, name="kt_sb")
            v_sb = pp.tile([P, NKT, DS], F32R, name="v_sb")
            ut_sb = pp.tile([P, HPC, S], F32R, name="ut_sb")
            onesk_sb = pp.tile([P, 1], F32R, name="onesk_sb")
            ones1_sb = pp.tile([1, P], F32R, name="ones1_sb")
            mask_sb = pp.tile([P, P], F32, name="mask_sb")

            # ---- load inputs ----
            nc.sync.dma_start(onesk_sb[:], ones_k[:])
            nc.sync.dma_start(ones1_sb[:], ones_1[:])
            nc.sync.dma_start(mask_sb[:], mask01[:])
            for w_sb, w_dram in (
                (wkT_sb, wkT),
                (wvT_sb, wvT),
                (wselfT_sb, wselfT),
            ):
                nc.sync.dma_start(
                    w_sb[:], w_dram.rearrange("(c p) n -> p c n", p=P)
                )
            nc.sync.dma_start(
                woutT_sb[:], woutT.rearrange("(c p) n -> p c n", p=P)
            )
            for c in range(NDC):
                nc.sync.dma_start(
                    xT_sb[:, c, :], xT[ts(c, P), :]
                )

            # ---- phase B: KT[h] = (Wk_h * hd^-0.25) @ x.T  [hd, S] ----
            for h in range(HPC):
                for j in range(NQC):
                    ps = ps_proj.tile([P, QC], F32, name="ps_b", tag="ps_proj")
                    for c in range(NDC):
                        nc.tensor.matmul(
                            ps[:],
                            wkT_sb[:, c, ts(h, HD)],
                            xT_sb[:, c, ts(j, QC)],
                            start=(c == 0),
                            stop=(c == NDC - 1),
                        )
                    nc.scalar.copy(kt_sb[:, h, ts(j, QC)], ps[:])

            # ---- phase C: V natural [S, 256] ----
            for st in range(NKT):
                ps = ps_proj.tile([P, QC], F32, name="ps_c", tag="ps_proj")
                for c in range(NDC):
                    nc.tensor.matmul(
                        ps[:, :DS],
                        xT_sb[:, c, ts(st, P)],
                        wvT_sb[:, c, :],
                        start=(c == 0),
                        stop=(c == NDC - 1),
                    )
                nc.scalar.copy(v_sb[:, st, :], ps[:, :DS])

            # ---- phase D: attention per (head, q-chunk) ----
            for h in range(HPC):
                for j in range(NQC):
                    nkt = 4 * j + 4  # causal: k tiles 0..4j+3
                    ot = ps_ot.tile([P, QC], F32, name="ot", tag="ps_ot")
                    lp = ps_l.tile([1, QC], F32, name="lp", tag="ps_l")
                    for kt in range(nkt):
                        c0 = max(0, P * kt - QC * j)
                        ep = ps_e.tile([P, QC], F32, name="ep", tag="ps_e")
                        # scores (k-part, q-free): E^T tile = KT[kt].T @ KT[qchunk]
                        nc.tensor.matmul(
                            ep[:, c0:],
                            kt_sb[:, h, ts(kt, P)],
                            kt_sb[:, h, QC * j + c0 : QC * (j + 1)],
                            start=True,
                            stop=True,
                        )
                        pt = pt_pool.tile([P, QC], F32R, name="pt", tag="pt")
                        nc.scalar.activation(pt[:, c0:], ep[:, c0:], EXP, scale=-1.0)
                        if kt >= 4 * j:
                            # diagonal subtile: zero disallowed (q < k)
                            nc.vector.tensor_mul(
                                pt[:, c0 : c0 + P],
                                pt[:, c0 : c0 + P].bitcast(F32),
                                mask_sb[:],
                            )
                        nc.tensor.matmul(
                            ot[:, c0:],
                            v_sb[:, kt, ts(h, HD)],
                            pt[:, c0:],
                            start=(kt == 0),
                            stop=(kt == nkt - 1),
                        )
                        nc.tensor.matmul(
                            lp[:, c0:],
                            onesk_sb[:],
                            pt[:, c0:],
                            start=(kt == 0),
                            stop=(kt == nkt - 1),
                        )
                    # normalize: UT[h, qchunk] = OT * broadcast(1/l)
                    rv = rb_pool.tile([1, QC], F32R, name="rv", tag="rv")
                    with nc.allow_low_precision("softmax denom to f32r"):
                        nc.vector.reciprocal(rv[:], lp[:])
                    bc = ps_bc.tile([P, QC], F32, name="bc", tag="ps_bc")
                    nc.tensor.matmul(bc[:], ones1_sb[:], rv[:], start=True, stop=True)
                    rb = rb_pool.tile([P, QC], F32, name="rb", tag="rb")
                    nc.scalar.copy(rb[:], bc[:])
                    nc.vector.tensor_mul(ut_sb[:, h, ts(j, QC)], ot[:], rb[:])

            # ---- phase E: UT += Wself_h @ x.T ----
            for m in range(HPC):
                for j in range(NQC):
                    ps = ps_proj.tile([P, QC], F32, name="ps_e2", tag="ps_proj")
                    for c in range(NDC):
                        nc.tensor.matmul(
                            ps[:],
                            wselfT_sb[:, c, ts(m, HD)],
                            xT_sb[:, c, ts(j, QC)],
                            start=(c == 0),
                            stop=(c == NDC - 1),
                        )
                    nc.vector.tensor_add(
                        ut_sb[:, m, ts(j, QC)],
                        ut_sb[:, m, ts(j, QC)].bitcast(F32),
                        ps[:],
                    )

            # ---- phase F: partial = U @ Wout.T slice ----
            for qt in range(NKT):
                ob = out_pool.tile([P, D], F32, name="ob", tag="ob")
                for nch in range(2):
                    ps = ps_proj.tile([P, QC], F32, name="ps_f", tag="ps_proj")
                    for m in range(HPC):
                        nc.tensor.matmul(
                            ps[:],
                            ut_sb[:, m, ts(qt, P)],
                            woutT_sb[:, m, ts(nch, QC)],
                            start=(m == 0),
                            stop=(m == HPC - 1),
                        )
                    if nch == 0:
                        nc.scalar.copy(ob[:, ts(nch, QC)], ps[:])
                    else:
                        nc.vector.tensor_copy(ob[:, ts(nch, QC)], ps[:])
                nc.sync.dma_start(part[ts(qt, P), :], ob[:])

    _legalize_waits(nc)
    return nc


_NC = None


def _get_nc():
    global _NC
    if _NC is None:
        _NC = _build()
    return _NC


def kernel(x, Wk, Wv, Wself, Wout):
    x = np.ascontiguousarray(np.asarray(x, dtype=np.float32))
    Wk = np.asarray(Wk, dtype=np.float32)
    Wv = np.asarray(Wv, dtype=np.float32)
    Wself = np.asarray(Wself, dtype=np.float32)
    Wout = np.asarray(Wout, dtype=np.float32)

    nc = _get_nc()

    kscale = np.float32(HD ** -0.25)
    xT = [np.ascontiguousarray(x[b].T) for b in range(B)]
    ones_k = np.ones((P, 1), np.float32)
    ones_1 = np.ones((1, P), np.float32)
    mask01 = np.triu(np.ones((P, P), np.float32))  # allow q >= k (free >= part)

    in_maps = []
    for c in range(N_CORES):
        b, hp = divmod(c, 4)
        ds = slice(DS * hp, DS * (hp + 1))
        in_maps.append(
            {
                "xT": xT[b],
                "wkT": np.ascontiguousarray((Wk[ds, :] * kscale).T),
                "wvT": np.ascontiguousarray(Wv[ds, :].T),
                "wselfT": np.ascontiguousarray(Wself[ds, :].T),
                "woutT": np.ascontiguousarray(Wout[:, ds].T),
                "ones_k": ones_k,
                "ones_1": ones_1,
                "mask01": mask01,
            }
        )

    res = run_bass_kernel_spmd(nc, in_maps, core_ids=list(range(N_CORES)))

    out = np.empty((B, S, D), np.float32)
    for b in range(B):
        acc = np.zeros((S, D), np.float64)
        for hp in range(4):
            acc += res.results[4 * b + hp]["part"]
        out[b] = acc.astype(np.float32)
    return out
